# revision 24
# baseline (speedup 1.0000x reference)
"""Additive-attention kernel for Trainium2 (8 NeuronCores, SPMD).

Problem (per batch b of B=4):
    xt      = x[b].T                                  # (N=512, D=96)
    g1      = xt @ Wg1.T                              # (512, 256)
    g2      = xt @ Wg2.T                              # (512, 256)
    score   = sum_a Wa[a] * tanh(g1[n,a] + g2[m,a] + bg[a])    # (512, 512)
    att     = sigmoid(score + Wa_b + ba)
    out[b]  = att @ xt                                # (512, 96)

Sharding: core c handles batch b = c//2 and query-rows n in
[(c%2)*256, (c%2)*256+256).  Each core computes its full out rows; the
host concatenates.

Algorithm (v6, odd-harmonic Fourier factorization): approximate
    tanh(u+v) ~= sum_{j in 1,3,5,7} BJ_j * sin(j*S*(u+v)),  S = pi/L
(weighted LSQ fit of tanh on |u+v|<=9; even-harmonic coefficients of
the optimal fit are ~0, so only odd harmonics are computed).  Each
harmonic separates, sin(jTu+jTv) = sin(jTu)cos(jTv) + cos(jTu)sin(jTv),
so the N x N score matrix becomes matmuls over a contraction dim of
(a, j, sin|cos).

Odd harmonics come from a step-2 Chebyshev recurrence with multiplier
2cos(2t):  f_j = 2cos(2t)*f_{j-2} - f_{j-4}, seeded by f_1 and a fused
j=3 step  f_3 = (s1*(2cos2+1), c1*(2cos2-1)) (one tensor_tensor with
the per-lane multiplier tile m3).  All recurrence work runs as fp16
tensor_tensor (2x DVE mode) / tensor_scalar (4x) ops; u and v sides
live in separate tiles so the u pipeline (theta -> seeds -> chain)
starts on the Vector engine while the Scalar engine still produces v
seeds, and the v chain is emitted as early as dependencies allow (it
gates the final score matmuls).  2cos(2t) = 4c^2-2 is squared on DVE
(ACT Square lives in a different LUT set).  Wa[a]*BJ_j feature scaling
runs on the Scalar engine in its idle window between seeds and the
sigmoid tail.  scalar_tensor_tensor fusion was tried: 1x mode only,
slower overall.

Startup: seed-Sin biases (0, pi/2) come from memset const tiles, not
the bias DMA, so seeds depend only on theta; xkT ships pre-swizzled to
the SBUF tile's natural [128, 4, 96] layout (128 large descriptors vs
512 small ones); the Wa-derived bias columns transfer last.

Scoring: per (j, fn, a-chunk, m-block) matmul with the v-side feature
block stationary -> scoreT[m, n] accumulates into 4 PSUM banks
[128, 256] fp32; the last harmonic runs m-block-major so sigmoids can
start early.  Sigmoid (+Wa_b+ba) PSUM->SBUF fp16 yields attT[m, n],
the lhsT of the final out[n, d] matmul against x[b].T (fp16).  The
output returns to HBM as fp16 (error contribution ~5e-4 rel, well
under the fit error) and is cast to fp32 on the host.
"""

import numpy as np

B, D, N, A = 4, 96, 512, 256
NH = N // 2          # query rows per core
NCORES = 8

JS = (1, 3, 5, 7)
FL = 11.0
FS = float(np.pi / FL)
# weighted-LSQ fit of tanh on |t|<=9, weights N(0,1.3^2)+0.01, basis
# sin(j*pi/11*t), j in {1,3,5,7}
BJ = {1: 1.23409, 3: 0.322111, 5: 0.108264, 7: 0.075567}

_cache = {}


def _build_nc_v6(bg_zero=False):
    import concourse.bacc as bacc
    import concourse.mybir as mybir
    from concourse import tile

    f32 = mybir.dt.float32
    f16 = mybir.dt.float16
    AF = mybir.ActivationFunctionType
    MULT = mybir.AluOpType.mult
    ADD = mybir.AluOpType.add

    nc = bacc.Bacc("TRN2", target_bir_lowering=False)

    NBC = 15 if not bg_zero else 11
    vin = nc.dram_tensor("vin", [D, A + N], f16, kind="ExternalInput")
    uin = nc.dram_tensor("uin", [D, A + NH], f16, kind="ExternalInput")
    biasv = nc.dram_tensor("biasv", [128, NBC], f32, kind="ExternalInput")
    xkTP = nc.dram_tensor("xkTP", [128, 4 * D], f16, kind="ExternalInput")
    out = nc.dram_tensor("out", [NH, D], f16, kind="ExternalOutput")

    FV = N * 2           # 1024: v-side feature width (keys x 2 a-chunks)
    FU = NH * 2          # 512:  u-side feature width

    with tile.TileContext(nc) as tc:
        with (
            tc.tile_pool(name="consts", bufs=1) as consts,
            tc.tile_pool(name="feat", bufs=1) as feat,
            tc.tile_pool(name="uscal", bufs=1) as uscal,
            tc.tile_pool(name="tmpp", bufs=2) as tmpp,
            tc.tile_pool(name="gps", bufs=1, space="PSUM") as gps,
            tc.tile_pool(name="scps", bufs=1, space="PSUM") as scps,
            tc.tile_pool(name="attp", bufs=1) as attp,
            tc.tile_pool(name="opool", bufs=1) as opool,
        ):
            vin_sb = consts.tile([D, A + N], f16, tag="vin")
            uin_sb = consts.tile([D, A + NH], f16, tag="uin")
            biasv_sb = consts.tile([128, NBC], f32, tag="biasv")
            xkT_sb = consts.tile([128, 4, D], f16, tag="xkT")
            w2_sb = vin_sb[:, :A]
            xk_sb = vin_sb[:, A:A + N]
            w1_sb = uin_sb[:, :A]
            xq_sb = uin_sb[:, A:A + NH]
            wav_sb = biasv_sb[:, 0:2]
            sgb_sb = biasv_sb[:, 2:3]
            wab_sb = {j: biasv_sb[:, 3 + 2 * i:5 + 2 * i]
                      for i, j in enumerate(JS)}
            if not bg_zero:
                bsin_sb = biasv_sb[:, 11:13]
                bcos_sb = biasv_sb[:, 13:15]

            # pi/2 const for the cos seeds + dummy Sin to preload the
            # ACT table set during the input DMAs
            hpi = consts.tile([128, 1], f32, tag="hpi")
            nc.vector.memset(hpi[:], float(np.pi / 2))
            dummy = consts.tile([128, 1], f32, tag="dummy")
            nc.vector.memset(dummy[:], 0.0)
            nc.scalar.activation(dummy[:], dummy[:], AF.Sin)

            # input DMAs split across both HWDGE queues, u-side pieces
            # first (theta_u gates the serial ACT seed phase), Wa bias
            # columns last (first consumer runs ~3us after data lands)
            if not bg_zero:
                nc.sync.dma_start(biasv_sb[:], biasv.ap())
            nc.sync.dma_start(uin_sb[:, :A], uin.ap()[:, :A])
            nc.scalar.dma_start(uin_sb[:, A:], uin.ap()[:, A:])
            nc.scalar.dma_start(vin_sb[:, :A], vin.ap()[:, :A])
            nc.gpsimd.dma_start(vin_sb[:, A:], vin.ap()[:, A:])
            nc.gpsimd.dma_start(xkT_sb[:], xkTP.ap())
            if bg_zero:
                nc.gpsimd.dma_start(biasv_sb[:], biasv.ap())

            # all-ones lhsT for the 0.5*colsum(x) accumulation (the
            # sigmoid -> 0.5+0.5*tanh rewrite's constant term)
            ones = consts.tile([128, 128], f16, tag="ones")
            nc.vector.memset(ones[:], 1.0)

            # PE warmup during the input-DMA wait: sustained matmul
            # activity ramps the tensor-engine DVFS clock (otherwise the
            # first ~40 real matmuls run at half speed)

            # theta = S*(g [+ bg via ACT bias]) per side, K=D=96 matmuls
            thu = gps.tile([128, FU], f32, tag="thu", name="thu")
            for c in range(2):
                nc.tensor.matmul(thu[:, c * NH:(c + 1) * NH],
                                 w1_sb[:, c * 128:(c + 1) * 128], xq_sb[:])
            thv = gps.tile([128, 2, N], f32, tag="thv", name="thv")
            for c in range(2):
                nc.tensor.matmul(thv[:, c, :],
                                 w2_sb[:, c * 128:(c + 1) * 128], xk_sb[:])

            # constant half-sum term: fos[nb] starts as sum_m xkT[m, :]
            # (the ones matmuls run early, right after xkT lands)
            fos = [gps.tile([128, D], f32, tag="fo", name=f"fo{nb}")
                   for nb in range(2)]
            for mb in range(4):
                for nb in range(2):
                    nc.tensor.matmul(
                        fos[nb][:], ones[:], xkT_sb[:, mb, :],
                        start=(mb == 0), stop=False, skip_group_check=True,
                    )

            # per-side feature tiles [128, 2(sin|cos), W]
            cu = {j: feat.tile([128, 2, FU], f16, tag=f"cu{j}", name=f"cu{j}")
                  for j in JS}
            cv = {j: feat.tile([128, 2, 2, N], f16, tag=f"cv{j}",
                               name=f"cv{j}") for j in JS}

            def seed_lane(cf1, th, W, lane):
                bias = hpi[:] if lane == 1 else 0.0
                if bg_zero:
                    nc.scalar.activation(cf1[:, lane], th[:], AF.Sin,
                                         bias=bias)
                else:
                    bl = bcos_sb if lane == 1 else bsin_sb
                    H = W // 2
                    for c in range(2):
                        nc.scalar.activation(cf1[:, lane, c * H:(c + 1) * H]
                                             if W == FU else cf1[:, lane, c],
                                             th[:, c * H:(c + 1) * H]
                                             if W == FU else th[:, c],
                                             AF.Sin, bias=bl[:, c:c + 1])

            # cos lanes first on ACT: the DVE setup (sq -> 2cos2t -> m3)
            # needs only cos; sin lanes are not consumed until j3
            seed_lane(cu[1], thu, FU, 1)
            seed_lane(cv[1], thv, FV, 1)
            seed_lane(cu[1], thu, FU, 0)
            seed_lane(cv[1], thv, FV, 0)

            # u-side Wa*BJ scaled features, all on ACT's idle window
            # between the seeds and the sigmoid tail
            us = {j: uscal.tile([128, 2, FU], f16, tag=f"us{j}", name=f"us{j}")
                  for j in JS}

            def uscale_act(j):
                for c in range(2):
                    nc.scalar.activation(
                        us[j][:, :, c * NH:(c + 1) * NH],
                        cu[j][:, :, c * NH:(c + 1) * NH],
                        AF.Identity, scale=wab_sb[j][:, c:c + 1])

            def uscale_dve(j):
                for c in range(2):
                    nc.vector.tensor_scalar(
                        us[j][:, :, c * NH:(c + 1) * NH],
                        cu[j][:, :, c * NH:(c + 1) * NH],
                        wav_sb[:, c:c + 1], float(BJ[j]), MULT, MULT)

            # DVE setup per side: sq = c^2 (tensor_tensor), then
            # 2cos(2t) = 4c^2-2 and m3 = (2cos2+1, 2cos2-1) at 4x mode
            def setup(cf1, W, tg):
                sq = feat.tile([128, W], f16, tag=f"sq{tg}", name=f"sq{tg}")
                t2 = feat.tile([128, 1, W], f16, tag=f"t2{tg}", name=f"t2{tg}")
                m3 = feat.tile([128, 2, W], f16, tag=f"m3{tg}", name=f"m3{tg}")
                nc.vector.tensor_mul(sq[:], cf1[:, 1, :], cf1[:, 1, :])
                nc.vector.tensor_scalar(t2[:, 0, :], sq[:], 4.0, -2.0,
                                        MULT, ADD)
                nc.vector.tensor_scalar(m3[:, 0, :], t2[:, 0, :], 1.0, 1.0,
                                        MULT, ADD)
                nc.vector.tensor_scalar(m3[:, 1, :], t2[:, 0, :], 1.0, -1.0,
                                        MULT, ADD)
                return t2[:, 0:1, :].broadcast_to((128, 2, W)), m3

            sc = [scps.tile([128, NH], f32, tag=f"sc{mb}", name=f"sc{mb}")
                  for mb in range(4)]

            def score_mms(j, first=False, last=False):
                loops = ([(mb, fn, c) for mb in range(4)
                          for fn in range(2) for c in range(2)] if last else
                         [(mb, fn, c) for fn in range(2)
                          for c in range(2) for mb in range(4)])
                for mb, fn, c in loops:
                    nc.tensor.matmul(
                        sc[mb][:],
                        cv[j][:, 1 - fn, c, mb * 128:(mb + 1) * 128],
                        us[j][:, fn, c * NH:(c + 1) * NH],
                        start=(first and fn == 0 and c == 0),
                        stop=(last and fn == 1 and c == 1),
                        skip_group_check=True,
                    )

            # DVE emission order (execution order): u pipeline and
            # v setup early; after j3v the v chain runs m-block-major
            # so score matmuls and the tanh tail pipeline behind it
            t2u_b, m3u = setup(cu[1], FU, "u")
            nc.vector.tensor_mul(cu[3][:], cu[1][:], m3u[:])
            tu = tmpp.tile([128, 2, FU], f16, tag="tu")
            nc.vector.tensor_mul(tu[:], cu[3][:], t2u_b)
            nc.vector.tensor_sub(cu[5][:], tu[:], cu[1][:])

            sqv = feat.tile([128, 2, N], f16, tag="sqv", name="sqv")
            t2v = feat.tile([128, 1, 2, N], f16, tag="t2v", name="t2v")
            m3v = feat.tile([128, 2, 2, N], f16, tag="m3v", name="m3v")
            nc.vector.tensor_mul(sqv[:], cv[1][:, 1], cv[1][:, 1])
            nc.vector.tensor_scalar(t2v[:, 0], sqv[:], 4.0, -2.0, MULT, ADD)
            nc.vector.tensor_scalar(m3v[:, 0], t2v[:, 0], 1.0, 1.0, MULT, ADD)
            nc.vector.tensor_scalar(m3v[:, 1], t2v[:, 0], 1.0, -1.0, MULT, ADD)
            t2v_b4 = t2v[:, 0:1].broadcast_to((128, 2, 2, N))
            nc.vector.tensor_mul(cv[3][:], cv[1][:], m3v[:])

            uscale_dve(1)
            uscale_act(3)
            uscale_act(5)

            score_mms(1, first=True)
            score_mms(3)

            tu2 = tmpp.tile([128, 2, FU], f16, tag="tu")
            nc.vector.tensor_mul(tu2[:], cu[5][:], t2u_b)
            nc.vector.tensor_sub(cu[7][:], tu2[:], cu[3][:])
            uscale_act(7)

            tv = tmpp.tile([128, 2, 2, N], f16, tag="tv")
            nc.vector.tensor_mul(tv[:], cv[3][:], t2v_b4)
            tv2 = tmpp.tile([128, 2, 2, N], f16, tag="tv")

            attT = attp.tile([128, 4, NH], f16, tag="attT")
            for mb in range(4):
                sl = slice(mb * 128, (mb + 1) * 128)
                nc.vector.tensor_sub(cv[5][:, :, :, sl], tv[:, :, :, sl],
                                     cv[1][:, :, :, sl])
                nc.vector.tensor_mul(tv2[:, :, :, sl], cv[5][:, :, :, sl],
                                     t2v_b4[:, :, :, sl])
                nc.vector.tensor_sub(cv[7][:, :, :, sl], tv2[:, :, :, sl],
                                     cv[3][:, :, :, sl])
                for j in (5, 7):
                    for fn in range(2):
                        for c in range(2):
                            nc.tensor.matmul(
                                sc[mb][:],
                                cv[j][:, 1 - fn, c, sl],
                                us[j][:, fn, c * NH:(c + 1) * NH],
                                start=False,
                                stop=(j == 7 and fn == 1 and c == 1),
                                skip_group_check=True,
                            )
                nc.scalar.activation(
                    attT[:, mb, :], sc[mb][:], AF.Tanh, scale=0.5,
                    bias=sgb_sb[:, 0:1]
                )
                for nb in range(2):
                    nc.tensor.matmul(
                        fos[nb][:],
                        attT[:, mb, nb * 128:(nb + 1) * 128],
                        xkT_sb[:, mb, :],
                        start=False,
                        stop=(mb == 3),
                        skip_group_check=True,
                    )

            out_sb = opool.tile([128, 2, D], f16, tag="out")
            for nb in range(2):
                nc.vector.tensor_scalar_mul(out_sb[:, nb, :], fos[nb][:], 0.5)
            nc.sync.dma_start(
                out.ap().rearrange("(nb p) d -> p nb d", p=128), out_sb[:]
            )

    nc.compile()
    return nc


def _prep_inputs_v6(x, Wg1, Wg2, bg, Wa_w, Wa_b, ba, bg_zero):
    """Host-side packing/slicing only (no reference math)."""
    x = np.asarray(x, np.float32)
    w1s = FS * np.asarray(Wg1, np.float32).T
    w2s = FS * np.asarray(Wg2, np.float32).T
    wac = np.asarray(Wa_w, np.float32).reshape(2, 128).T
    NBC = 11 if bg_zero else 15
    biasv = np.empty((128, NBC), np.float32)
    biasv[:, 0:2] = wac
    biasv[:, 2] = 0.5 * (float(np.asarray(Wa_b).ravel()[0])
                         + float(np.asarray(ba).ravel()[0]))
    for i, j in enumerate(JS):
        biasv[:, 3 + 2 * i:5 + 2 * i] = wac * np.float32(BJ[j])
    if not bg_zero:
        bgv = FS * np.asarray(bg, np.float32)
        biasv[:, 11:13] = bgv.reshape(2, 128).T
        biasv[:, 13:15] = bgv.reshape(2, 128).T + np.float32(np.pi / 2)
    in_maps = []
    for c in range(NCORES):
        b, half = c // 2, c % 2
        xb = x[b]
        vin = np.ascontiguousarray(
            np.concatenate([w2s, xb], axis=1), dtype=np.float16)
        uin = np.ascontiguousarray(
            np.concatenate([w1s, xb[:, half * NH:(half + 1) * NH]], axis=1),
            dtype=np.float16)
        xkTP = np.ascontiguousarray(
            xb.T.astype(np.float16).reshape(4, 128, D)
            .transpose(1, 0, 2).reshape(128, 4 * D))
        in_maps.append({
            "vin": vin,
            "uin": uin,
            "biasv": np.ascontiguousarray(biasv),
            "xkTP": xkTP,
        })
    return in_maps


def _run(inputs, trace=False):
    from concourse.bass_utils import run_bass_kernel_spmd

    bg_zero = bool(np.all(np.asarray(inputs["bg"]) == 0))
    key = ("nc6", bg_zero)
    if key not in _cache:
        _cache[key] = _build_nc_v6(bg_zero=bg_zero)
    nc = _cache[key]
    in_maps = _prep_inputs_v6(**inputs, bg_zero=bg_zero)
    res = run_bass_kernel_spmd(
        nc, in_maps, core_ids=list(range(NCORES)), trace=trace
    )
    out = np.empty((B, N, D), np.float32)
    for c in range(NCORES):
        b, half = c // 2, c % 2
        out[b, half * NH:(half + 1) * NH] = \
            res.results[c]["out"].astype(np.float32)
    return out, res


def kernel(**inputs):
    out, _ = _run(inputs, trace=False)
    return out


# revision 25
# speedup vs baseline: 1.0172x; 1.0172x over previous
"""Additive-attention kernel for Trainium2 (8 NeuronCores, SPMD).

Problem (per batch b of B=4):
    xt      = x[b].T                                  # (N=512, D=96)
    g1      = xt @ Wg1.T                              # (512, 256)
    g2      = xt @ Wg2.T                              # (512, 256)
    score   = sum_a Wa[a] * tanh(g1[n,a] + g2[m,a] + bg[a])    # (512, 512)
    att     = sigmoid(score + Wa_b + ba)
    out[b]  = att @ xt                                # (512, 96)

Sharding: core c handles batch b = c//2 and query-rows n in
[(c%2)*256, (c%2)*256+256).  Each core computes its full out rows; the
host concatenates.

Algorithm (v6, odd-harmonic Fourier factorization): approximate
    tanh(u+v) ~= sum_{j in 1,3,5,7} BJ_j * sin(j*S*(u+v)),  S = pi/L
(weighted LSQ fit of tanh on |u+v|<=9; even-harmonic coefficients of
the optimal fit are ~0, so only odd harmonics are computed).  Each
harmonic separates, sin(jTu+jTv) = sin(jTu)cos(jTv) + cos(jTu)sin(jTv),
so the N x N score matrix becomes matmuls over a contraction dim of
(a, j, sin|cos).

Odd harmonics come from a step-2 Chebyshev recurrence with multiplier
2cos(2t):  f_j = 2cos(2t)*f_{j-2} - f_{j-4}, seeded by f_1 and a fused
j=3 step  f_3 = (s1*(2cos2+1), c1*(2cos2-1)) (one tensor_tensor with
the per-lane multiplier tile m3).  All recurrence work runs as fp16
tensor_tensor (2x DVE mode) / tensor_scalar (4x) ops; u and v sides
live in separate tiles so the u pipeline (theta -> seeds -> chain)
starts on the Vector engine while the Scalar engine still produces v
seeds, and the v chain is emitted as early as dependencies allow (it
gates the final score matmuls).  2cos(2t) = 4c^2-2 is squared on DVE
(ACT Square lives in a different LUT set).  Wa[a]*BJ_j feature scaling
runs on the Scalar engine in its idle window between seeds and the
sigmoid tail.  scalar_tensor_tensor fusion was tried: 1x mode only,
slower overall.

Startup: seed-Sin biases (0, pi/2) come from memset const tiles, not
the bias DMA, so seeds depend only on theta; xkT ships pre-swizzled to
the SBUF tile's natural [128, 4, 96] layout (128 large descriptors vs
512 small ones); the Wa-derived bias columns transfer last.

Scoring: per (j, fn, a-chunk, m-block) matmul with the v-side feature
block stationary -> scoreT[m, n] accumulates into 4 PSUM banks
[128, 256] fp32; the last harmonic runs m-block-major so sigmoids can
start early.  Sigmoid (+Wa_b+ba) PSUM->SBUF fp16 yields attT[m, n],
the lhsT of the final out[n, d] matmul against x[b].T (fp16).  The
output returns to HBM as fp16 (error contribution ~5e-4 rel, well
under the fit error) and is cast to fp32 on the host.
"""

import numpy as np

B, D, N, A = 4, 96, 512, 256
NH = N // 2          # query rows per core
NCORES = 8

JS = (1, 3, 5, 7)
FL = 11.0
FS = float(np.pi / FL)
# weighted-LSQ fit of tanh on |t|<=9, weights N(0,1.3^2)+0.01, basis
# sin(j*pi/11*t), j in {1,3,5,7}
BJ = {1: 1.23409, 3: 0.322111, 5: 0.108264, 7: 0.075567}

_cache = {}


def _build_nc_v6(bg_zero=False):
    import concourse.bacc as bacc
    import concourse.mybir as mybir
    from concourse import tile

    f32 = mybir.dt.float32
    f16 = mybir.dt.float16
    AF = mybir.ActivationFunctionType
    MULT = mybir.AluOpType.mult
    ADD = mybir.AluOpType.add

    nc = bacc.Bacc("TRN2", target_bir_lowering=False)

    NBC = 15 if not bg_zero else 11
    vin = nc.dram_tensor("vin", [D, A + N], f16, kind="ExternalInput")
    uin = nc.dram_tensor("uin", [D, A + NH], f16, kind="ExternalInput")
    biasv = nc.dram_tensor("biasv", [128, NBC], f32, kind="ExternalInput")
    xkTP = nc.dram_tensor("xkTP", [128, 4 * D], f16, kind="ExternalInput")
    out = nc.dram_tensor("out", [NH, D], f16, kind="ExternalOutput")

    FV = N * 2           # 1024: v-side feature width (keys x 2 a-chunks)
    FU = NH * 2          # 512:  u-side feature width

    with tile.TileContext(nc) as tc:
        with (
            tc.tile_pool(name="consts", bufs=1) as consts,
            tc.tile_pool(name="feat", bufs=1) as feat,
            tc.tile_pool(name="uscal", bufs=1) as uscal,
            tc.tile_pool(name="tmpp", bufs=2) as tmpp,
            tc.tile_pool(name="gps", bufs=1, space="PSUM") as gps,
            tc.tile_pool(name="scps", bufs=1, space="PSUM") as scps,
            tc.tile_pool(name="attp", bufs=1) as attp,
            tc.tile_pool(name="opool", bufs=1) as opool,
        ):
            vin_sb = consts.tile([D, A + N], f16, tag="vin")
            uin_sb = consts.tile([D, A + NH], f16, tag="uin")
            biasv_sb = consts.tile([128, NBC], f32, tag="biasv")
            xkT_sb = consts.tile([128, 4, D], f16, tag="xkT")
            w2_sb = vin_sb[:, :A]
            xk_sb = vin_sb[:, A:A + N]
            w1_sb = uin_sb[:, :A]
            xq_sb = uin_sb[:, A:A + NH]
            wav_sb = biasv_sb[:, 0:2]
            sgb_sb = biasv_sb[:, 2:3]
            wab_sb = {j: biasv_sb[:, 3 + 2 * i:5 + 2 * i]
                      for i, j in enumerate(JS)}
            if not bg_zero:
                bsin_sb = biasv_sb[:, 11:13]
                bcos_sb = biasv_sb[:, 13:15]

            # pi/2 const for the cos seeds + dummy Sin to preload the
            # ACT table set during the input DMAs
            hpi = consts.tile([128, 1], f32, tag="hpi")
            nc.vector.memset(hpi[:], float(np.pi / 2))
            dummy = consts.tile([128, 1], f32, tag="dummy")
            nc.vector.memset(dummy[:], 0.0)
            nc.scalar.activation(dummy[:], dummy[:], AF.Sin)

            # input DMAs split across both HWDGE queues, u-side pieces
            # first (theta_u gates the serial ACT seed phase), Wa bias
            # columns last (first consumer runs ~3us after data lands)
            if not bg_zero:
                nc.sync.dma_start(biasv_sb[:], biasv.ap())
            nc.sync.dma_start(uin_sb[:, :A], uin.ap()[:, :A])
            nc.scalar.dma_start(uin_sb[:, A:], uin.ap()[:, A:])
            nc.scalar.dma_start(vin_sb[:, :A], vin.ap()[:, :A])
            nc.gpsimd.dma_start(vin_sb[:, A:], vin.ap()[:, A:])
            nc.gpsimd.dma_start(xkT_sb[:], xkTP.ap())
            if bg_zero:
                nc.gpsimd.dma_start(biasv_sb[:], biasv.ap())

            # all-ones lhsT for the 0.5*colsum(x) accumulation (the
            # sigmoid -> 0.5+0.5*tanh rewrite's constant term)
            ones = consts.tile([128, 128], f16, tag="ones")
            nc.vector.memset(ones[:], 1.0)

            # PE warmup during the input-DMA wait: sustained matmul
            # activity ramps the tensor-engine DVFS clock (otherwise the
            # first ~40 real matmuls run at half speed)

            # theta = S*(g [+ bg via ACT bias]) per side, K=D=96 matmuls
            thu = gps.tile([128, FU], f32, tag="thu", name="thu")
            for c in range(2):
                nc.tensor.matmul(thu[:, c * NH:(c + 1) * NH],
                                 w1_sb[:, c * 128:(c + 1) * 128], xq_sb[:])
            thv = gps.tile([128, 2, N], f32, tag="thv", name="thv")
            for c in range(2):
                nc.tensor.matmul(thv[:, c, :],
                                 w2_sb[:, c * 128:(c + 1) * 128], xk_sb[:])

            # constant half-sum term: fos[nb] starts as sum_m xkT[m, :]
            # (the ones matmuls run early, right after xkT lands)
            fos = [gps.tile([128, D], f32, tag="fo", name=f"fo{nb}")
                   for nb in range(2)]
            for mb in range(4):
                for nb in range(2):
                    nc.tensor.matmul(
                        fos[nb][:], ones[:], xkT_sb[:, mb, :],
                        start=(mb == 0), stop=False, skip_group_check=True,
                    )

            # per-side feature tiles [128, 2(sin|cos), W]
            cu = {j: feat.tile([128, 2, FU], f16, tag=f"cu{j}", name=f"cu{j}")
                  for j in JS}
            cv = {j: feat.tile([128, 2, 2, N], f16, tag=f"cv{j}",
                               name=f"cv{j}") for j in JS}

            def seed_lane(cf1, th, W, lane):
                bias = hpi[:] if lane == 1 else 0.0
                if bg_zero:
                    nc.scalar.activation(cf1[:, lane], th[:], AF.Sin,
                                         bias=bias)
                else:
                    bl = bcos_sb if lane == 1 else bsin_sb
                    H = W // 2
                    for c in range(2):
                        nc.scalar.activation(cf1[:, lane, c * H:(c + 1) * H]
                                             if W == FU else cf1[:, lane, c],
                                             th[:, c * H:(c + 1) * H]
                                             if W == FU else th[:, c],
                                             AF.Sin, bias=bl[:, c:c + 1])

            # cos lanes first on ACT: the DVE setup (sq -> 2cos2t -> m3)
            # needs only cos; sin lanes are not consumed until j3
            seed_lane(cu[1], thu, FU, 1)
            seed_lane(cv[1], thv, FV, 1)
            seed_lane(cu[1], thu, FU, 0)
            seed_lane(cv[1], thv, FV, 0)

            # u-side Wa*BJ scaled features, all on ACT's idle window
            # between the seeds and the sigmoid tail
            us = {j: uscal.tile([128, 2, FU], f16, tag=f"us{j}", name=f"us{j}")
                  for j in JS}

            def uscale_act(j):
                for c in range(2):
                    nc.scalar.activation(
                        us[j][:, :, c * NH:(c + 1) * NH],
                        cu[j][:, :, c * NH:(c + 1) * NH],
                        AF.Identity, scale=wab_sb[j][:, c:c + 1])

            def uscale_dve(j):
                for c in range(2):
                    nc.vector.tensor_scalar(
                        us[j][:, :, c * NH:(c + 1) * NH],
                        cu[j][:, :, c * NH:(c + 1) * NH],
                        wav_sb[:, c:c + 1], float(BJ[j]), MULT, MULT)

            # DVE setup per side: sq = c^2 (tensor_tensor), then
            # 2cos(2t) = 4c^2-2 and m3 = (2cos2+1, 2cos2-1) at 4x mode
            def setup(cf1, W, tg):
                sq = feat.tile([128, W], f16, tag=f"sq{tg}", name=f"sq{tg}")
                t2 = feat.tile([128, 1, W], f16, tag=f"t2{tg}", name=f"t2{tg}")
                m3 = feat.tile([128, 2, W], f16, tag=f"m3{tg}", name=f"m3{tg}")
                nc.vector.tensor_mul(sq[:], cf1[:, 1, :], cf1[:, 1, :])
                nc.vector.tensor_scalar(t2[:, 0, :], sq[:], 4.0, -2.0,
                                        MULT, ADD)
                nc.vector.tensor_scalar(m3[:, 0, :], t2[:, 0, :], 1.0, 1.0,
                                        MULT, ADD)
                nc.vector.tensor_scalar(m3[:, 1, :], t2[:, 0, :], 1.0, -1.0,
                                        MULT, ADD)
                return t2[:, 0:1, :].broadcast_to((128, 2, W)), m3

            sc = [scps.tile([128, NH], f32, tag=f"sc{mb}", name=f"sc{mb}")
                  for mb in range(4)]

            def score_mms(j, first=False, last=False):
                loops = ([(mb, fn, c) for mb in range(4)
                          for fn in range(2) for c in range(2)] if last else
                         [(mb, fn, c) for fn in range(2)
                          for c in range(2) for mb in range(4)])
                for mb, fn, c in loops:
                    nc.tensor.matmul(
                        sc[mb][:],
                        cv[j][:, 1 - fn, c, mb * 128:(mb + 1) * 128],
                        us[j][:, fn, c * NH:(c + 1) * NH],
                        start=(first and fn == 0 and c == 0),
                        stop=(last and fn == 1 and c == 1),
                        skip_group_check=True,
                    )

            # DVE emission order (execution order): u pipeline and
            # v setup early; after j3v the v chain runs m-block-major
            # so score matmuls and the tanh tail pipeline behind it
            t2u_b, m3u = setup(cu[1], FU, "u")
            nc.vector.tensor_mul(cu[3][:], cu[1][:], m3u[:])
            tu = tmpp.tile([128, 2, FU], f16, tag="tu")
            nc.vector.tensor_mul(tu[:], cu[3][:], t2u_b)
            nc.vector.tensor_sub(cu[5][:], tu[:], cu[1][:])

            sqv = feat.tile([128, 2, N], f16, tag="sqv", name="sqv")
            t2v = feat.tile([128, 1, 2, N], f16, tag="t2v", name="t2v")
            m3v = feat.tile([128, 2, 2, N], f16, tag="m3v", name="m3v")
            nc.vector.tensor_mul(sqv[:], cv[1][:, 1], cv[1][:, 1])
            nc.vector.tensor_scalar(t2v[:, 0], sqv[:], 4.0, -2.0, MULT, ADD)
            nc.vector.tensor_scalar(m3v[:, 0], t2v[:, 0], 1.0, 1.0, MULT, ADD)
            nc.vector.tensor_scalar(m3v[:, 1], t2v[:, 0], 1.0, -1.0, MULT, ADD)
            t2v_b4 = t2v[:, 0:1].broadcast_to((128, 2, 2, N))
            nc.vector.tensor_mul(cv[3][:], cv[1][:], m3v[:])

            uscale_dve(1)
            uscale_act(3)
            uscale_act(5)

            score_mms(1, first=True)
            score_mms(3)

            tu2 = tmpp.tile([128, 2, FU], f16, tag="tu")
            nc.vector.tensor_mul(tu2[:], cu[5][:], t2u_b)
            nc.vector.tensor_sub(cu[7][:], tu2[:], cu[3][:])
            uscale_act(7)

            tv = tmpp.tile([128, 2, 2, N], f16, tag="tv")
            tv2 = tmpp.tile([128, 2, 2, N], f16, tag="tv")

            attT = attp.tile([128, 4, NH], f16, tag="attT")
            for mb in range(4):
                sl = slice(mb * 128, (mb + 1) * 128)
                nc.vector.tensor_mul(tv[:, :, :, sl], cv[3][:, :, :, sl],
                                     t2v_b4[:, :, :, sl])
                nc.vector.tensor_sub(cv[5][:, :, :, sl], tv[:, :, :, sl],
                                     cv[1][:, :, :, sl])
                nc.vector.tensor_mul(tv2[:, :, :, sl], cv[5][:, :, :, sl],
                                     t2v_b4[:, :, :, sl])
                nc.vector.tensor_sub(cv[7][:, :, :, sl], tv2[:, :, :, sl],
                                     cv[3][:, :, :, sl])
                for j in (5, 7):
                    for fn in range(2):
                        for c in range(2):
                            nc.tensor.matmul(
                                sc[mb][:],
                                cv[j][:, 1 - fn, c, sl],
                                us[j][:, fn, c * NH:(c + 1) * NH],
                                start=False,
                                stop=(j == 7 and fn == 1 and c == 1),
                                skip_group_check=True,
                            )
                nc.scalar.activation(
                    attT[:, mb, :], sc[mb][:], AF.Tanh, scale=0.5,
                    bias=sgb_sb[:, 0:1]
                )
                for nb in range(2):
                    nc.tensor.matmul(
                        fos[nb][:],
                        attT[:, mb, nb * 128:(nb + 1) * 128],
                        xkT_sb[:, mb, :],
                        start=False,
                        stop=(mb == 3),
                        skip_group_check=True,
                    )

            out_sb = opool.tile([128, 2, D], f16, tag="out")
            for nb in range(2):
                nc.vector.tensor_scalar_mul(out_sb[:, nb, :], fos[nb][:], 0.5)
            nc.sync.dma_start(
                out.ap().rearrange("(nb p) d -> p nb d", p=128), out_sb[:]
            )

    nc.compile()
    return nc


def _prep_inputs_v6(x, Wg1, Wg2, bg, Wa_w, Wa_b, ba, bg_zero):
    """Host-side packing/slicing only (no reference math)."""
    x = np.asarray(x, np.float32)
    w1s = FS * np.asarray(Wg1, np.float32).T
    w2s = FS * np.asarray(Wg2, np.float32).T
    wac = np.asarray(Wa_w, np.float32).reshape(2, 128).T
    NBC = 11 if bg_zero else 15
    biasv = np.empty((128, NBC), np.float32)
    biasv[:, 0:2] = wac
    biasv[:, 2] = 0.5 * (float(np.asarray(Wa_b).ravel()[0])
                         + float(np.asarray(ba).ravel()[0]))
    for i, j in enumerate(JS):
        biasv[:, 3 + 2 * i:5 + 2 * i] = wac * np.float32(BJ[j])
    if not bg_zero:
        bgv = FS * np.asarray(bg, np.float32)
        biasv[:, 11:13] = bgv.reshape(2, 128).T
        biasv[:, 13:15] = bgv.reshape(2, 128).T + np.float32(np.pi / 2)
    in_maps = []
    for c in range(NCORES):
        b, half = c // 2, c % 2
        xb = x[b]
        vin = np.ascontiguousarray(
            np.concatenate([w2s, xb], axis=1), dtype=np.float16)
        uin = np.ascontiguousarray(
            np.concatenate([w1s, xb[:, half * NH:(half + 1) * NH]], axis=1),
            dtype=np.float16)
        xkTP = np.ascontiguousarray(
            xb.T.astype(np.float16).reshape(4, 128, D)
            .transpose(1, 0, 2).reshape(128, 4 * D))
        in_maps.append({
            "vin": vin,
            "uin": uin,
            "biasv": np.ascontiguousarray(biasv),
            "xkTP": xkTP,
        })
    return in_maps


def _run(inputs, trace=False):
    from concourse.bass_utils import run_bass_kernel_spmd

    bg_zero = bool(np.all(np.asarray(inputs["bg"]) == 0))
    key = ("nc6", bg_zero)
    if key not in _cache:
        _cache[key] = _build_nc_v6(bg_zero=bg_zero)
    nc = _cache[key]
    in_maps = _prep_inputs_v6(**inputs, bg_zero=bg_zero)
    res = run_bass_kernel_spmd(
        nc, in_maps, core_ids=list(range(NCORES)), trace=trace
    )
    out = np.empty((B, N, D), np.float32)
    for c in range(NCORES):
        b, half = c // 2, c % 2
        out[b, half * NH:(half + 1) * NH] = \
            res.results[c]["out"].astype(np.float32)
    return out, res


def kernel(**inputs):
    out, _ = _run(inputs, trace=False)
    return out
